# revision 32
# baseline (speedup 1.0000x reference)
"""Trainium2 Bass kernel for nn_BasicTransDecoderBlock (dense_transformer).

v3: data-parallel over batch B=8 across 8 NeuronCores (1 sample/core).
Key structure vs v2 baseline:
 - All 9 depthwise taps of every 3x3 conv run on the TensorEngine as
   diagonal-weight matmuls PSUM-accumulated over shifted padded views;
   Scalar evacuates. Diag tiles built on-device from eye * tap-weight.
 - BN1 affine algebraically folded into the pointwise weights (scale on
   the contract dim) plus a 9-pattern border-bias correction applied as
   a tiny extra matmul accumulated into the same PSUM group (contract=9
   over pattern indicator maps). No per-map affine passes.
 - Inputs shipped bf16 and DMA'd straight into the padded image layout;
   BN stats from Scalar accum (x2 squares) / DVE reduce+bn_stats.
 - Front-loaded DMA slimmed (no dwod/selb consts) so the first
   collective's mesh barrier isn't stuck behind input loads.
 - LN-q deferral kept; the per-(head,px) rs/mrs stats are broadcast to
   all 32 channels of each head by stride-0 DMA (idle DMA engines)
   instead of select-matmul + PSUM evac; mrs*( -Gg)+BVX prefolded into
   one M map so the per-block work is 1 DVE mul + 1 GpSimd add.
 - attn out = Q'(K'^T V')/d + (bias @ V')/d reassociation kept.

Self-contained: hardcodes all shapes; imports only the concourse runtime.
"""
import sys
import numpy as np
import ml_dtypes

for _p in ("/opt/trn_rl_repo", "/root/.axon_site/_ro/trn_rl_repo"):
    if _p not in sys.path:
        sys.path.insert(0, _p)

import concourse.bass as bass
import concourse.bacc as bacc
import concourse.tile as tile
from concourse import mybir
from concourse.bass_utils import run_bass_kernel_spmd

FP32 = mybir.dt.float32
BF16 = mybir.dt.bfloat16
ALU = mybir.AluOpType
ACTF = mybir.ActivationFunctionType

B, IN_CH, OUT_CH, HEADS, DIM_HEAD, R = 8, 512, 256, 8, 32, 16
H1, W1, H2, W2 = 32, 32, 64, 64
EPS_BN, EPS_LN = 1e-5, 1e-6
N1, N2, NS = H1 * W1, H2 * W2, R * R     # 1024, 4096, 256
P = 128
NCORES = 8
PW1, PW2 = W1 + 2, W2 + 2                # padded widths 34, 66
PAD1, PAD2 = (H1 + 2) * PW1, (H2 + 2) * PW2   # 1156, 4356
TAPS = [(dy, dx) for dy in range(3) for dx in range(3)]


# ---------------------------------------------------------------- host helpers

def _interp_matrix(n_in, n_out):
    A = np.zeros((n_out, n_in), np.float32)
    xs = np.linspace(0.0, n_in - 1.0, n_out)
    for i, x in enumerate(xs):
        x0 = int(np.floor(x)); x1 = min(x0 + 1, n_in - 1)
        w = x - x0
        A[i, x0] += 1.0 - w
        A[i, x1] += w
    return A


def _head_major_perm():
    perm = np.zeros(OUT_CH, np.int64)
    for h in range(HEADS):
        for d in range(DIM_HEAD):
            perm[h * DIM_HEAD + d] = d * HEADS + h
    return perm


def _rel_bias_small(rel_table):
    c = np.stack(np.meshgrid(np.arange(R), np.arange(R), indexing="ij")).reshape(2, -1)
    rel = (c[:, :, None] - c[:, None, :]).transpose(1, 2, 0)
    rel[:, :, 0] += R - 1
    rel[:, :, 1] += R - 1
    rel[:, :, 0] *= 2 * R - 1
    idx = rel.sum(-1).reshape(-1)
    return np.asarray(rel_table, np.float32)[idx].reshape(NS, NS, HEADS)


def _r64_chunks():
    """Residue resize (32->64), ch-major: per 512-pixel output chunk only a
    few 128-pixel input tiles contribute."""
    Ay, Ax = _interp_matrix(H1, H2), _interp_matrix(W1, W2)
    R64 = np.kron(Ay, Ax).astype(np.float32)       # [4096, 1024]
    ktiles, blocks = [], []
    for nn in range(8):
        rows = R64[nn * 512:(nn + 1) * 512]
        used = [kk for kk in range(8)
                if np.abs(rows[:, kk * 128:(kk + 1) * 128]).sum() > 0]
        ktiles.append(used)
        for kk in used:
            blocks.append(rows[:, kk * 128:(kk + 1) * 128].T.copy())
    return ktiles, np.concatenate(blocks, axis=0)


_R64_KTILES, _R64_PACKED = _r64_chunks()
_N_R64_SLOTS = sum(len(k) for k in _R64_KTILES)


def _etab(dw):
    """[C, 9] border-pattern coefficients so that the padded-conv constant
    response kappa = sum_pat etab[:,pat] * P_pat with patterns
    [ones, T, B, L, R, TL, TR, BL, BR]."""
    C = dw.shape[0]
    e = np.zeros((C, 9), np.float32)
    e[:, 0] = dw.sum(1)
    e[:, 1] = -(dw[:, 0] + dw[:, 1] + dw[:, 2])
    e[:, 2] = -(dw[:, 6] + dw[:, 7] + dw[:, 8])
    e[:, 3] = -(dw[:, 0] + dw[:, 3] + dw[:, 6])
    e[:, 4] = -(dw[:, 2] + dw[:, 5] + dw[:, 8])
    e[:, 5] = dw[:, 0]
    e[:, 6] = dw[:, 2]
    e[:, 7] = dw[:, 6]
    e[:, 8] = dw[:, 8]
    return e


def _patterns(H, W):
    Pm = np.zeros((9, H, W), np.float32)
    Pm[0] = 1.0
    Pm[1, 0, :] = 1.0
    Pm[2, H - 1, :] = 1.0
    Pm[3, :, 0] = 1.0
    Pm[4, :, W - 1] = 1.0
    Pm[5, 0, 0] = 1.0
    Pm[6, 0, W - 1] = 1.0
    Pm[7, H - 1, 0] = 1.0
    Pm[8, H - 1, W - 1] = 1.0
    return Pm.reshape(9, H * W)


def _selb():
    selb = np.zeros((16 * P, OUT_CH), np.float32)
    hh = np.arange(OUT_CH) // DIM_HEAD
    for blk in range(16):
        for h in range(HEADS):
            # stats row packing (set by the relayout DMA stream order):
            # row = 16*(blk//2) + 2h + (blk%2)
            selb[blk * P + 16 * (blk // 2) + 2 * h + (blk % 2), :] = (hh == h)
    return selb


def _host_prep(inp):
    perm = _head_major_perm()
    f32 = lambda a: np.ascontiguousarray(np.asarray(a, np.float32))
    bf = lambda a: np.ascontiguousarray(np.asarray(a, np.float32).astype(ml_dtypes.bfloat16))

    kvw = np.asarray(inp["to_kv_pw"], np.float32).reshape(2 * OUT_CH, IN_CH)
    gq = np.asarray(inp["normq_g"], np.float32).reshape(-1)   # head-major (h,d)
    wq = np.asarray(inp["to_q_pw"], np.float32).reshape(OUT_CH, OUT_CH)[perm].T
    wq = wq * gq[None, :]                                     # fold g into WQ
    hh = np.arange(OUT_CH) // DIM_HEAD
    # SELP[c, (mm,s,b), 2h+bb]: head-select/g^(s+1), gated to b==bb and the
    # channels of out-tile mm; accumulating the 4 (mm,b) matmuls of chunk nn
    # writes the packed stats rows 2h+b directly.
    selp = np.zeros((OUT_CH, 8, 32), np.float32)
    base = np.equal(hh[:, None], np.arange(HEADS)[None, :]).astype(np.float32)
    for s in range(2):
        g = base / (gq[:, None] ** (s + 1))
        for par in range(2):
            for b in range(2):
                sel = np.zeros((OUT_CH, 32), np.float32)
                sel[:, 16 * par + b::2][:, 0:HEADS] = g
                selp[:, 4 * s + 2 * par + b, :] = sel

    dw1 = np.asarray(inp["to_kv_dw"], np.float32).reshape(IN_CH, 9)
    dwq = np.asarray(inp["to_q_dw"], np.float32).reshape(OUT_CH, 9)
    dwo = np.asarray(inp["to_out_dw"], np.float32).reshape(OUT_CH, 9)[perm]

    rt16 = np.kron(_interp_matrix(H1, R), _interp_matrix(W1, R)).astype(np.float32)

    d = {
        "wch": bf(np.asarray(inp["conv_ch_w"], np.float32).reshape(OUT_CH, IN_CH).T),
        "wkv": bf(np.concatenate([kvw[perm].T, kvw[OUT_CH + perm].T], axis=1)),
        "wq": bf(wq),
        "wout": bf(np.asarray(inp["to_out_pw"], np.float32).reshape(OUT_CH, OUT_CH)[:, perm].T),
        "wmlp": bf(np.asarray(inp["mlp_w"], np.float32).reshape(OUT_CH, OUT_CH).T),
        "dw1w": f32(dw1),
        "dwqw": f32(dwq),
        "dwow": f32(dwo),
        "e1tab": f32(_etab(dw1)),
        "e2tab": f32(_etab(dwq)),
        "p16t": bf((rt16 @ _patterns(H1, W1).T).T),      # [9, 256]
        "p2m": bf(np.concatenate([_patterns(H2, W2)[:, 0:512],
                                  _patterns(H2, W2)[:, 512:1024],
                                  _patterns(H2, W2)[:, 3584:4096]], axis=1)),
        "idm": bf(np.eye(P, dtype=np.float32)),
        "rt16": bf(rt16.T),                              # [1024, 256]
        "r64c": bf(_R64_PACKED),
        "selp": bf(selp.reshape(OUT_CH, 8 * 32)),
        "bvt": bf(_rel_bias_small(inp["rel_table"]).transpose(2, 1, 0)
                  .reshape(HEADS * NS, NS)),
        "gkb": bf(np.tile(np.asarray(inp["normk_g"], np.float32).reshape(1, OUT_CH), (P, 1))),
        "bkb": bf(np.tile(np.asarray(inp["normk_b"], np.float32).reshape(1, OUT_CH), (P, 1))),
        "gqb": bf(gq.reshape(OUT_CH, 1)),
        "selb": bf(_selb()),
        "bqb": bf(np.asarray(inp["normq_b"], np.float32).reshape(OUT_CH, 1)),
    }
    pk = np.zeros((P, 18), np.float32)
    pk[:, 0:4] = np.asarray(inp["norm_l_g"], np.float32).reshape(4, P).T
    pk[:, 4:8] = np.asarray(inp["norm_l_b"], np.float32).reshape(4, P).T
    pk[:, 8:10] = np.asarray(inp["norm_h_g"], np.float32).reshape(2, P).T
    pk[:, 10:12] = np.asarray(inp["norm_h_b"], np.float32).reshape(2, P).T
    pk[:, 12:14] = np.asarray(inp["norm2_g"], np.float32).reshape(2, P).T
    pk[:, 14:16] = np.asarray(inp["norm2_b"], np.float32).reshape(2, P).T
    pk[:, 16:18] = np.asarray(inp["conv_ch_b"], np.float32).reshape(2, P).T
    d["bnpk"] = pk
    return d


# ---------------------------------------------------------------- device build

def _img(ap, w):
    return ap.rearrange("p (h w) -> p h w", w=w)


def _pad_memsets(nc, xpad, Hs, Ws, pw):
    """Zero only the pad cells of a [p, (Hs+2)*pw] image buffer (GpSimd)."""
    nc.gpsimd.memset(xpad[:, 0:pw + 1], 0.0)
    mid = xpad[:, pw + Ws + 1:pw + Ws + 1 + (Hs - 1) * pw] \
        .rearrange("p (h c) -> p h c", c=pw)[:, :, 0:2]
    nc.gpsimd.memset(mid, 0.0)
    nc.gpsimd.memset(xpad[:, (Hs + 1) * pw - 1:(Hs + 2) * pw], 0.0)


def _emit_dw_pe9(nc, ps, diag, dslot, xpad, Hs, Ws, pw, dst, name):
    """All 9 taps on the TensorEngine: diagonal-weight matmuls accumulated
    in PSUM over shifted padded-image views; Scalar evacuates each
    512-px chunk into dst (bf16)."""
    nch = (Hs * Ws) // 512
    rpc = 512 // Ws
    xv = _img(xpad, pw)
    for g0 in range(0, nch, 4):
        cs = list(range(g0, min(g0 + 4, nch)))
        accs = [ps.tile([P, 512], FP32, tag="mm512", name=f"{name}{c}")
                for c in cs]
        for i in range(9):
            dy, dx = TAPS[i]
            for a, c in zip(accs, cs):
                nc.tensor.matmul(a[:], diag[:, dslot + i, :],
                                 xv[:, dy + rpc * c:dy + rpc * c + rpc,
                                    dx:dx + Ws],
                                 start=(i == 0), stop=(i == 8))
        for a, c in zip(accs, cs):
            nc.scalar.activation(dst[:, bass.ts(c, 512)], a[:],
                                 ACTF.Identity, bias=0.0)


def _emit(nc, tc, dram, out_d):
    import contextlib
    ctx = contextlib.ExitStack()
    pool = lambda name, bufs, space="SBUF": ctx.enter_context(
        tc.tile_pool(name=name, bufs=bufs, space=space))

    consts = pool("consts", 1)
    work = pool("work", 1)
    imgs = pool("imgs", 1)       # big image slabs with tag reuse
    tr4 = pool("tr4", 2)
    ps = pool("ps", 5, "PSUM")
    pss = pool("pss", 2, "PSUM")
    psd = pool("psd", 1, "PSUM")
    dpool = pool("dramp", 1, "DRAM")

    dma = nc.sync.dma_start

    # ---------------- tiny tables first (gate DIAG builds + stats math)
    def load_c(name, shape, dt=FP32, q=None):
        t = consts.tile(shape, dt, tag=name)
        src = dram[name].ap()
        if len(shape) == 3 and shape[0] == P:
            src = src.rearrange("(t p) n -> p t n", p=P)
        elif len(shape) == 3:
            src = src.rearrange("p (t n) -> p t n", t=shape[1])
        (q or dma)(t[:], src)
        return t

    vq = nc.scalar.dma_start
    DW1W = load_c("dw1w", [P, 4, 9], q=vq)
    DWQW = load_c("dwqw", [P, 2, 9], q=vq)
    DWOW = load_c("dwow", [P, 2, 9], q=vq)
    E1T = load_c("e1tab", [P, 4, 9], q=vq)
    E2T = load_c("e2tab", [P, 2, 9], q=vq)
    BNPK = load_c("bnpk", [P, 18], q=vq)
    IDM = load_c("idm", [P, P], BF16, q=vq)
    SELP = load_c("selp", [P, 2, 256], BF16, q=vq)
    GQB = load_c("gqb", [P, 2, 1], BF16, q=vq)
    BQB = load_c("bqb", [P, 2, 1], BF16, q=vq)
    P16T = load_c("p16t", [9, 2, P], BF16, q=vq)
    P2M = load_c("p2m", [9, 3, 512], BF16, q=vq)
    GKB = load_c("gkb", [P, OUT_CH], BF16, q=vq)
    BKB = load_c("bkb", [P, OUT_CH], BF16, q=vq)

    # ---------------- x1/x2 (bf16) flat loads; padded copies on Scalar
    X1B = imgs.tile([P, 4, N1], BF16, tag="X1B")
    x1src = dram["x1b"].ap().rearrange("(t p) n -> p t n", p=P)
    for t in range(4):
        dma(X1B[:, t, :], x1src[:, t, :])
    X2B = imgs.tile([P, 2, N2], BF16, tag="X2B")
    x2src = dram["x2b"].ap().rearrange("(t p) n -> p t n", p=P)
    for t in range(2):
        dma(X2B[:, t, :], x2src[:, t, :])
    WCH = load_c("wch", [P, 4, OUT_CH], BF16)
    XP1 = [imgs.tile([P, PAD1], BF16, tag=f"XP1_{t}", name=f"XP1_{t}")
           for t in range(4)]
    for t in range(4):
        _pad_memsets(nc, XP1[t][:], H1, W1, PW1)
        nc.scalar.activation(_img(XP1[t][:], PW1)[:, 1:1 + H1, 1:1 + W1],
                             _img(X1B[:, t, :], W1), ACTF.Identity, bias=0.0)
    XP2 = imgs.tile([P, 2, PAD2], BF16, tag="bigA")
    for t in range(2):
        _pad_memsets(nc, XP2[:, t, :], H2, W2, PW2)
    WKV = load_c("wkv", [P, 4, 2 * OUT_CH], BF16)
    WQ = load_c("wq", [P, 2, OUT_CH], BF16)
    RT16 = load_c("rt16", [P, 8, NS], BF16)
    WOUT = load_c("wout", [P, 2, OUT_CH], BF16)
    WMLP = load_c("wmlp", [P, 2, OUT_CH], BF16)
    SELB = consts.tile([P, 16, OUT_CH], BF16, tag="selb")
    BVT = consts.tile([P, 2 * HEADS, NS], BF16, tag="bvt")

    # ---------------- PE warm-keeper scratch (HAM clock-gate)
    WKS = work.tile([P, 1024], BF16, tag="WKS")
    WKT = work.tile([P, 64], BF16, tag="WKT")
    nc.gpsimd.memset(WKS[:], 0.0)
    nc.gpsimd.memset(WKT[:], 0.0)

    def warm_chain(n, tag):
        # paced dummy links: matmul -> DVE copy -> matmul ... spreads PE
        # activity across dependency-stall windows to hold the HAM at 8/8
        acc = psd.tile([P, 512], FP32, tag="psdum", name=f"wc_{tag}")
        for k in range(n):
            nc.tensor.matmul(acc[0:64, 0:64], WKT[:], WKS[:, 0:64],
                             start=True, stop=True)
            nc.vector.tensor_copy(WKT[0:64, :], acc[0:64, 0:64])

    dumacc = psd.tile([P, 512], FP32, tag="psdum", name="dum0")
    for _ in range(8):
        nc.tensor.matmul(dumacc[:], WKS[:, 0:P], WKS[:, 0:512],
                         start=True, stop=True)

    # ---------------- DIAG tap tiles (DVE, from eye * per-channel weight)
    DIAG1 = work.tile([P, 36, P], BF16, tag="slabB")
    DIAGQ = work.tile([P, 18, P], BF16, tag="slabC")
    DIAGO = imgs.tile([P, 18, P], BF16, tag="XP1_0")
    for t in range(4):
        for i in range(9):
            nc.vector.tensor_scalar(DIAG1[:, 9 * t + i, :], IDM[:],
                                    DW1W[:, t, i:i + 1], None, ALU.mult)
    for t in range(2):
        for i in range(9):
            nc.vector.tensor_scalar(DIAGQ[:, 9 * t + i, :], IDM[:],
                                    DWQW[:, t, i:i + 1], None, ALU.mult)

    # ---------------- conv_ch transposed (for the residue) - evac on Scalar
    X1CT = work.tile([P, 8, OUT_CH], BF16, tag="X1CT")
    for m in range(8):
        acc = ps.tile([P, 512], FP32, tag="mm512", name=f"cch{m}")
        for kk in range(4):
            nc.tensor.matmul(acc[:, 0:OUT_CH], X1B[:, kk, bass.ts(m, P)],
                             WCH[:, kk, :], start=(kk == 0), stop=(kk == 3))
        nc.scalar.activation(X1CT[:, m, :], acc[:, 0:OUT_CH],
                             ACTF.Identity, bias=0.0)

    # ---------------- BN1 stats -> ccin (cols: x1 S/S2 x4, x2 S x2, x2 S2 x2)
    ccin = work.tile([P, 12], FP32, tag="ccin")
    st1 = work.tile([P, 4, 12], FP32, tag="st1")
    ag1 = work.tile([P, 4, 2], FP32, tag="ag1")
    for t in range(4):
        for c in range(2):
            nc.vector.bn_stats(st1[:, t, 6 * c:6 * c + 6], X1B[:, t, bass.ts(c, 512)])
        nc.vector.bn_aggr(ag1[:, t, :],
                          st1[:, t, :].rearrange("p (c s) -> p c s", s=6))
        m, v = ag1[:, t, 0:1], ag1[:, t, 1:2]
        S, S2 = ccin[:, 2 * t:2 * t + 1], ccin[:, 2 * t + 1:2 * t + 2]
        nc.vector.tensor_scalar(S, m, float(N1), None, ALU.mult)
        nc.vector.tensor_mul(S2, m, m)
        nc.vector.tensor_add(S2, S2, v)
        nc.vector.tensor_scalar(S2, S2, float(N1), None, ALU.mult)
    # x2: bn_stats on the flat copy (DVE); XP2 interior builds stay pure
    st2x = work.tile([P, 2, 48], FP32, tag="st2x")
    ag2x = work.tile([P, 2, 2], FP32, tag="ag2x")
    for t in range(2):
        for c in range(8):
            nc.vector.bn_stats(st2x[:, t, 6 * c:6 * c + 6],
                               X2B[:, t, bass.ts(c, 512)])
        nc.vector.bn_aggr(ag2x[:, t, :],
                          st2x[:, t, :].rearrange("p (c s) -> p c s", s=6))
        m, v = ag2x[:, t, 0:1], ag2x[:, t, 1:2]
        S, S2 = ccin[:, 8 + t:9 + t], ccin[:, 10 + t:11 + t]
        nc.vector.tensor_scalar(S, m, float(N2), None, ALU.mult)
        nc.vector.tensor_mul(S2, m, m)
        nc.vector.tensor_add(S2, S2, v)
        nc.vector.tensor_scalar(S2, S2, float(N2), None, ALU.mult)
    for t in range(2):
        nc.scalar.activation(_img(XP2[:, t, :], PW2)[:, 1:1 + H2, 1:1 + W2],
                             _img(X2B[:, t, :], W2), ACTF.Identity, bias=0.0)

    cc1i = dpool.tile([P, 12], FP32, tag="cc1i")
    cc1o = dpool.tile([P, 12], FP32, tag="cc1o")
    nc.gpsimd.dma_start(cc1i[:], ccin[:])
    nc.gpsimd.collective_compute("AllReduce", ALU.add,
                                 replica_groups=[list(range(NCORES))],
                                 ins=[cc1i.opt()], outs=[cc1o.opt()])
    ccout = work.tile([P, 12], FP32, tag="ccout")
    nc.gpsimd.dma_start(ccout[:], cc1o[:])

    # ---------------- raw depthwise on PE (overlaps AllReduce flight)
    DW1 = imgs.tile([P, 4, N1], BF16, tag="DW1")
    for t in range(4):
        _emit_dw_pe9(nc, ps, DIAG1, 9 * t, XP1[t][:], H1, W1, PW1,
                     DW1[:, t, :], f"dw1_{t}_")
    DWQ = imgs.tile([P, 2, N2], BF16, tag="bigB")
    for t in range(2):
        _emit_dw_pe9(nc, ps, DIAGQ, 9 * t, XP2[:, t, :], H2, W2, PW2,
                     DWQ[:, t, :], f"dwq_{t}_")
    dma(BVT[:], dram["bvt"].ap().rearrange("(t p) n -> p t n", p=P))
    dma(SELB[:], dram["selb"].ap().rearrange("(t p) n -> p t n", p=P))
    # DWO diag build (DVE has slack during the collective flight)
    for t in range(2):
        for i in range(9):
            nc.vector.tensor_scalar(DIAGO[:, 9 * t + i, :], IDM[:],
                                    DWOW[:, t, i:i + 1], None, ALU.mult)

    # ---------------- BN sandwich -> bnS/bnT ; fold into weights + u-tables
    bnS = work.tile([P, 6], FP32, tag="bnS")
    bnT = work.tile([P, 6], FP32, tag="bnT")
    mean6 = work.tile([P, 6], FP32, tag="mean6")
    var6 = work.tile([P, 6], FP32, tag="var6")
    for t in range(6):
        n = float(B * (N1 if t < 4 else N2))
        if t < 4:
            S, S2 = ccout[:, 2 * t:2 * t + 1], ccout[:, 2 * t + 1:2 * t + 2]
        else:
            S, S2 = ccout[:, 4 + t:5 + t], ccout[:, 6 + t:7 + t]
        m, v = mean6[:, t:t + 1], var6[:, t:t + 1]
        nc.vector.tensor_scalar(m, S, 1.0 / n, None, ALU.mult)
        nc.vector.scalar_tensor_tensor(v, m, -1.0, m, ALU.mult, ALU.mult)
        nc.vector.scalar_tensor_tensor(v, S2, 1.0 / n, v, ALU.mult, ALU.add)
        nc.vector.tensor_scalar(v, v, EPS_BN, None, ALU.add)
    nc.vector.reciprocal(var6[:], var6[:])
    nc.scalar.activation(bnS[:], var6[:], ACTF.Sqrt)
    nc.vector.tensor_mul(bnS[:, 0:4], bnS[:, 0:4], BNPK[:, 0:4])
    nc.vector.tensor_mul(bnS[:, 4:6], bnS[:, 4:6], BNPK[:, 8:10])
    nc.vector.tensor_mul(mean6[:], mean6[:], bnS[:])
    nc.vector.tensor_sub(bnT[:, 0:4], BNPK[:, 4:8], mean6[:, 0:4])
    nc.vector.tensor_sub(bnT[:, 4:6], BNPK[:, 10:12], mean6[:, 4:6])

    # u tables: U1[pat, o] = sum_c t1[c] e1[c,pat] WKV[c,o]  (pre-fold WKV)
    TE1 = work.tile([P, 4, 9], BF16, tag="TE1")
    TE2 = work.tile([P, 2, 9], BF16, tag="TE2")
    for t in range(4):
        nc.vector.tensor_scalar(TE1[:, t, :], E1T[:, t, :],
                                bnT[:, t:t + 1], None, ALU.mult)
    for t in range(2):
        nc.vector.tensor_scalar(TE2[:, t, :], E2T[:, t, :],
                                bnT[:, 4 + t:5 + t], None, ALU.mult)
    U1 = work.tile([9, 2 * OUT_CH], BF16, tag="U1")
    UQ9 = work.tile([9, 2, P], BF16, tag="UQ9")
    u1acc = pss.tile([P, 512], FP32, tag="psmall", name="u1acc")
    for kk in range(4):
        nc.tensor.matmul(u1acc[0:9, :], TE1[:, kk, :], WKV[:, kk, :],
                         start=(kk == 0), stop=(kk == 3))
    nc.scalar.copy(U1[:], u1acc[0:9, :])
    uqacc = pss.tile([P, 512], FP32, tag="psmall", name="uqacc")
    for kk in range(2):
        nc.tensor.matmul(uqacc[0:9, 0:OUT_CH], TE2[:, kk, :], WQ[:, kk, :],
                         start=(kk == 0), stop=(kk == 1))
    nc.scalar.copy(UQ9[:].rearrange("p m c -> p (m c)"), uqacc[0:9, 0:OUT_CH])
    # fold BN scale into the pointwise weights (in place; WAR-safe)
    for kk in range(4):
        nc.vector.tensor_scalar(WKV[:, kk, :], WKV[:, kk, :],
                                bnS[:, kk:kk + 1], None, ALU.mult)
    for t in range(2):
        nc.vector.tensor_scalar(WQ[:, t, :], WQ[:, t, :],
                                bnS[:, 4 + t:5 + t], None, ALU.mult)

    # ---------------- kv pointwise (pixel-major out), Scalar evac
    KVT = imgs.tile([P, 8, 2 * OUT_CH], BF16, tag="big16")
    for m in range(8):
        acc = ps.tile([P, 512], FP32, tag="mm512", name=f"kv{m}")
        for kk in range(4):
            nc.tensor.matmul(acc[:], DW1[:, kk, bass.ts(m, P)], WKV[:, kk, :],
                             start=(kk == 0), stop=(kk == 3))
        nc.scalar.activation(KVT[:, m, :], acc[:], ACTF.Identity, bias=0.0)

    # resize 32->16 + pattern bias: KVS = RT16^T @ KVT + p16t^T @ U1
    KVS = []
    for mm in range(2):
        acc = pss.tile([P, 512], FP32, tag="psmall", name=f"kvs{mm}")
        for kk in range(8):
            nc.tensor.matmul(acc[:], RT16[:, kk, bass.ts(mm, P)], KVT[:, kk, :],
                             start=(kk == 0), stop=False)
        nc.tensor.matmul(acc[:], P16T[:, mm, :], U1[:],
                         start=False, stop=True)
        KVS.append(acc)

    # LN-k + evac k' ; v' plain evac (bf16)
    KP = work.tile([P, 2, OUT_CH], BF16, tag="KP")
    VP = work.tile([P, 2, OUT_CH], BF16, tag="VP")
    ksq = work.tile([P, OUT_CH], FP32, tag="ksq")
    ksum = work.tile([P, HEADS], FP32, tag="ksum")
    km = work.tile([P, HEADS], FP32, tag="km")
    krs = work.tile([P, HEADS], FP32, tag="krs")
    kfp = work.tile([P, OUT_CH], FP32, tag="kfp")
    for mm in range(2):
        k_ap = KVS[mm][:, 0:OUT_CH].rearrange("p (h d) -> p h d", d=DIM_HEAD)
        nc.vector.tensor_reduce(ksum[:], k_ap, mybir.AxisListType.X, ALU.add,
                                opt_input=False)
        nc.scalar.activation(ksq[:], KVS[mm][:, 0:OUT_CH], ACTF.Square)
        nc.vector.tensor_reduce(krs[:], ksq[:].rearrange("p (h d) -> p h d",
                                                         d=DIM_HEAD),
                                mybir.AxisListType.X, ALU.add, opt_input=False)
        nc.vector.scalar_tensor_tensor(km[:], ksum[:], -1.0 / DIM_HEAD, ksum[:],
                                       ALU.mult, ALU.mult)
        nc.vector.tensor_add(krs[:], krs[:], km[:])
        nc.vector.tensor_scalar(krs[:], krs[:], DIM_HEAD * EPS_LN, None, ALU.add)
        nc.vector.reciprocal(krs[:], krs[:])
        nc.scalar.activation(krs[:], krs[:], ACTF.Sqrt, scale=float(DIM_HEAD))
        nc.vector.tensor_scalar(km[:], ksum[:], 1.0 / DIM_HEAD, None, ALU.mult)
        kb = km[:].unsqueeze(2).broadcast_to([P, HEADS, DIM_HEAD])
        rb = krs[:].unsqueeze(2).broadcast_to([P, HEADS, DIM_HEAD])
        t1 = kfp[:].rearrange("p (h d) -> p h d", d=DIM_HEAD)
        nc.vector.tensor_sub(t1, k_ap, kb)
        nc.vector.tensor_mul(t1, t1, rb)
        nc.vector.tensor_mul(kfp[:], kfp[:], GKB[:])
        nc.vector.tensor_add(KP[:, mm, :], kfp[:], BKB[:])
        nc.vector.tensor_copy(VP[:, mm, :], KVS[mm][:, OUT_CH:2 * OUT_CH])

    # A = K'^T V' / 32 : diagonal head blocks packed block-diagonal
    BD = work.tile([P, 2, P], BF16, tag="BD")
    nc.gpsimd.memset(BD[:], 0.0)
    for mo in range(2):
        acc = pss.tile([P, 512], FP32, tag="psmall", name=f"bd{mo}")
        for kk in range(2):
            nc.tensor.matmul(acc[:, 0:OUT_CH], KP[:, kk, bass.ts(mo, P)],
                             VP[:, kk, :], start=(kk == 0), stop=(kk == 1))
        for hh in range(4):
            h = mo * 4 + hh
            nc.scalar.activation(BD[bass.ds(32 * hh, 32), mo, bass.ds(32 * hh, 32)],
                                 acc[bass.ds(32 * hh, 32), bass.ds(32 * h, 32)],
                                 ACTF.Copy, scale=1.0 / DIM_HEAD)

    # Bb = BD @ b_q, Gg = BD @ g (per-channel consts for deferred LN-q)
    BbGg = work.tile([P, 2, 2], FP32, tag="BbGg")   # [:, pk, 0]=Bb, 1=-Gg
    for pk in range(2):
        acc = pss.tile([P, 512], FP32, tag="psmall", name=f"bbgg{pk}")
        nc.tensor.matmul(acc[:, 0:1], BD[:, pk, :], BQB[:, pk, :],
                         start=True, stop=True)
        nc.tensor.matmul(acc[:, 1:2], BD[:, pk, :], GQB[:, pk, :],
                         start=True, stop=True)
        nc.scalar.copy(BbGg[:, pk, 0:1], acc[:, 0:1])
        nc.scalar.activation(BbGg[:, pk, 1:2], acc[:, 1:2], ACTF.Identity,
                             bias=0.0, scale=-1.0)

    # BV[(h,d'), is] = (v'^T bias_small_h)/32 via full-M matmul + row extract
    BVC = work.tile([P, 2, NS], BF16, tag="BVC")
    for h in range(HEADS):
        mo, hh = h // 4, h % 4
        acc = pss.tile([P, 512], FP32, tag="psmall", name=f"bv{h}")
        for kk in range(2):
            nc.tensor.matmul(acc[:, 0:NS], VP[:, kk, bass.ts(mo, P)],
                             BVT[:, 2 * h + kk, :], start=(kk == 0), stop=(kk == 1))
        nc.scalar.activation(BVC[bass.ds(32 * hh, 32), mo, :],
                             acc[bass.ds(32 * hh, 32), 0:NS],
                             ACTF.Copy, scale=1.0 / DIM_HEAD)
    # expand along x: BVX[c, ys*64 + x] = BVC[c, ys*16 + x//4]; then += Bb
    BVX = work.tile([P, 2, R * W2], BF16, tag="BVX")
    for mo in range(2):
        nc.vector.tensor_copy(
            BVX[:, mo, :].rearrange("p (ys xs xr) -> p ys xs xr", xs=R, xr=4),
            BVC[:, mo, :].rearrange("p (ys xs) -> p ys xs", xs=R)
            .unsqueeze(3).broadcast_to([P, R, R, 4]))
        nc.vector.tensor_scalar(BVX[:, mo, :], BVX[:, mo, :],
                                BbGg[:, mo, 0:1], None, ALU.add)

    # ---------------- q pointwise (+pattern bias matmul) + LN-q stats
    Q = imgs.tile([P, 2, N2], BF16, tag="bigC")
    QSP = work.tile([P, 2, NS], FP32, tag="QSP")   # [(16nn+2h+b), (qs|q2s), 256]
    selv = SELP[:].rearrange("p m (v c) -> p m v c", c=32)
    for np_ in range(4):
        q2c = tr4.tile([P, 2, 2, 512], BF16, tag="tr4")
        for par in range(2):
            nn = 2 * np_ + par
            for mm in range(2):
                acc = ps.tile([P, 512], FP32, tag="mm512", name=f"q{nn}_{mm}")
                for kk in range(2):
                    nc.tensor.matmul(acc[:], WQ[:, kk, bass.ts(mm, P)],
                                     DWQ[:, kk, bass.ts(nn, 512)],
                                     start=(kk == 0), stop=False)
                nc.tensor.matmul(acc[:], UQ9[:, mm, :],
                                 P2M[:, 0 if nn == 0 else (2 if nn == 7 else 1), :],
                                 start=False, stop=True)
                nc.scalar.activation(Q[:, mm, bass.ts(nn, 512)], acc[:],
                                     ACTF.Identity, bias=0.0)
                nc.vector.tensor_mul(q2c[:, par, mm, :],
                                     Q[:, mm, bass.ts(nn, 512)],
                                     Q[:, mm, bass.ts(nn, 512)])
        for s in range(2):
            sacc = pss.tile([P, 512], FP32, tag="psmall", name=f"sel{np_}_{s}")
            j = 0
            for par in range(2):
                nn = 2 * np_ + par
                for mm in range(2):
                    for b in range(2):
                        if s == 0:
                            rhs = Q[:, mm, bass.ds(512 * nn + NS * b, NS)]
                        else:
                            rhs = q2c[:, par, mm, bass.ds(NS * b, NS)]
                        nc.tensor.matmul(
                            sacc[0:32, 0:NS],
                            selv[:, mm, 4 * s + 2 * par + b, :],
                            rhs, start=(j == 0), stop=(j == 7))
                        j += 1
            nc.vector.tensor_copy(QSP[bass.ds(32 * np_, 32), s, :],
                                  sacc[0:32, 0:NS])

    # rs | mrs  (bf16, packed)
    RSP = work.tile([P, 2, NS], BF16, tag="RSP")
    numt = work.tile([P, NS], FP32, tag="ksq")
    qsv, q2v = QSP[:, 0, :], QSP[:, 1, :]
    nc.vector.scalar_tensor_tensor(numt[:], qsv, -1.0 / DIM_HEAD, qsv, ALU.mult, ALU.mult)
    nc.vector.tensor_add(numt[:], numt[:], q2v)
    nc.vector.tensor_scalar(numt[:], numt[:], DIM_HEAD * EPS_LN, None, ALU.add)
    nc.vector.reciprocal(numt[:], numt[:])
    nc.scalar.activation(RSP[:, 0, :], numt[:], ACTF.Sqrt, scale=float(DIM_HEAD))
    nc.vector.scalar_tensor_tensor(RSP[:, 1, :], qsv, 1.0 / DIM_HEAD, RSP[:, 0, :],
                                   ALU.mult, ALU.mult)

    # ---------------- per-256-px block: broadcast stats via SELB matmul,
    # BD matmul, deferred LN affine, add bias map -> OPAD
    OPAD = imgs.tile([P, 2, PAD2], BF16, tag="bigA")   # reuses XP2 slab
    for t in range(2):
        _pad_memsets(nc, OPAD[:, t, :], H2, W2, PW2)
    # residue-resize table load (needed at WOUT time)
    R64C = work.tile([P, _N_R64_SLOTS, 512], BF16, tag="slabB")
    dma(R64C[:], dram["r64c"].ap().rearrange("(t p) n -> p t n", p=P))
    DWO = imgs.tile([P, 2, N2], BF16, tag="bigB")   # reuses DWQ slab

    def _dwo_chunk(t, c):
        # one 512-px to_out depthwise chunk: 9 diag-tap matmuls into one
        # PSUM bank + Scalar evac; interleaved into the block loop as soon
        # as its OPAD halo rows (blocks <= 2c+2) exist
        xv = _img(OPAD[:, t, :], PW2)
        a = ps.tile([P, 512], FP32, tag="mm512", name=f"dwo{t}_{c}")
        for i in range(9):
            dy, dx = TAPS[i]
            nc.tensor.matmul(a[:], DIAGO[:, 9 * t + i, :],
                             xv[:, dy + 8 * c:dy + 8 * c + 8, dx:dx + W2],
                             start=(i == 0), stop=(i == 8))
        nc.scalar.activation(DWO[:, t, bass.ts(c, 512)], a[:],
                             ACTF.Identity, bias=0.0)

    _DWO_SCHED = {4: (0, 0), 5: (1, 0), 6: (0, 1), 7: (1, 1), 8: (0, 2),
                  9: (1, 2), 10: (0, 3), 11: (1, 3), 12: (0, 4), 13: (1, 4),
                  14: (0, 5), 15: (1, 5)}
    rsp_flat = RSP[:].rearrange("p s f -> p (s f)")
    dumacc4 = psd.tile([P, 512], FP32, tag="psdum", name="dum4")
    for blk in range(16):
        # PE-only gap filler: keeps the HAM activity window busy between
        # the per-block matmul bursts (no cross-engine links)
        nc.tensor.matmul(dumacc4[:], WKS[:, 0:P], WKS[:, 0:512],
                         start=True, stop=True)
        rsb = tr4.tile([P, 2, 2, NS], BF16, tag="tr4")
        for mm in range(2):
            bacc = pss.tile([P, 512], FP32, tag="psmall", name=f"rb{blk}_{mm}")
            nc.tensor.matmul(bacc[:], SELB[:, blk, bass.ts(mm, P)], rsp_flat,
                             start=True, stop=True)
            nc.scalar.copy(rsb[:, mm, :, :],
                           bacc[:].rearrange("p (s f) -> p s f", f=NS))
        for pk in range(2):
            acc = ps.tile([P, 512], FP32, tag="mm512", name=f"at{blk}_{pk}")
            nc.tensor.matmul(acc[:, 0:NS], BD[:, pk, :],
                             Q[:, pk, bass.ds(blk * NS, NS)],
                             start=True, stop=True)
            tmpo = tr4.tile([P, NS], BF16, tag="tr4b")
            nc.vector.tensor_mul(tmpo[:], acc[:, 0:NS], rsb[:, pk, 0, :])
            nc.vector.scalar_tensor_tensor(tmpo[:], rsb[:, pk, 1, :],
                                           BbGg[:, pk, 1:2], tmpo[:],
                                           ALU.mult, ALU.add)
            dst = _img(OPAD[:, pk, :], PW2)[:, 1 + 4 * blk:5 + 4 * blk, 1:1 + W2]
            bv = BVX[:, pk, bass.ds(blk * W2, W2)].unsqueeze(1) \
                .broadcast_to([P, 4, W2])
            nc.gpsimd.tensor_add(dst,
                                 tmpo[:].rearrange("p (yr w) -> p yr w", w=W2),
                                 bv)
        if blk in _DWO_SCHED:
            _dwo_chunk(*_DWO_SCHED[blk])

    for t, c in ((0, 6), (1, 6), (0, 7), (1, 7)):
        _dwo_chunk(t, c)
    OSB = imgs.tile([P, 2, N2], BF16, tag="X1B")
    st2 = work.tile([P, 2, 48], FP32, tag="st2")
    ag2 = work.tile([P, 2, 2], FP32, tag="ag2")
    slot = 0
    for nn in range(8):
        used = _R64_KTILES[nn]
        for mm in range(2):
            acc = ps.tile([P, 512], FP32, tag="mm512", name=f"wo{nn}_{mm}")
            for kk in range(2):
                nc.tensor.matmul(acc[:], WOUT[:, kk, bass.ts(mm, P)],
                                 DWO[:, kk, bass.ts(nn, 512)],
                                 start=(kk == 0), stop=False)
            for i, kk in enumerate(used):
                s = slot + i
                nc.tensor.matmul(acc[:], X1CT[:, kk, bass.ts(mm, P)],
                                 R64C[:, s, :],
                                 start=False, stop=(i == len(used) - 1))
            nc.scalar.activation(OSB[:, mm, bass.ts(nn, 512)], acc[:],
                                 ACTF.Identity, bias=BNPK[:, 16 + mm:17 + mm])
            nc.vector.bn_stats(st2[:, mm, 6 * nn:6 * nn + 6],
                               OSB[:, mm, bass.ts(nn, 512)])
        slot += len(used)

    # ---------------- BN2 stats reduce + AllReduce
    cc2s = work.tile([P, 4], FP32, tag="cc2s")
    for t in range(2):
        nc.vector.bn_aggr(ag2[:, t, :],
                          st2[:, t, :].rearrange("p (c s) -> p c s", s=6))
        m, v = ag2[:, t, 0:1], ag2[:, t, 1:2]
        S, S2 = cc2s[:, 2 * t:2 * t + 1], cc2s[:, 2 * t + 1:2 * t + 2]
        nc.vector.tensor_scalar(S, m, float(N2), None, ALU.mult)
        nc.vector.tensor_mul(S2, m, m)
        nc.vector.tensor_add(S2, S2, v)
        nc.vector.tensor_scalar(S2, S2, float(N2), None, ALU.mult)
    cc2i = dpool.tile([P, 4], FP32, tag="cc2i")
    cc2o = dpool.tile([P, 4], FP32, tag="cc2o")
    nc.gpsimd.dma_start(cc2i[:], cc2s[:])
    nc.gpsimd.collective_compute("AllReduce", ALU.add,
                                 replica_groups=[list(range(NCORES))],
                                 ins=[cc2i.opt()], outs=[cc2o.opt()])
    cc2r = work.tile([P, 4], FP32, tag="cc2r")
    nc.gpsimd.dma_start(cc2r[:], cc2o[:])
    nc.vector.tensor_copy(WKT[:, 0:4], cc2r[:])
    dumacc3 = psd.tile([P, 512], FP32, tag="psdum", name="dum2")
    for _ in range(14):
        nc.tensor.matmul(dumacc3[:], WKS[:, 0:P], WKS[:, 0:512],
                         start=True, stop=True)
    bn3S = work.tile([P, 2], FP32, tag="bn3S")
    bn3T = work.tile([P, 2], FP32, tag="bn3T")
    m3 = work.tile([P, 2], FP32, tag="m3")
    v3 = work.tile([P, 2], FP32, tag="v3")
    nB = float(B * N2)
    for t in range(2):
        S, S2 = cc2r[:, 2 * t:2 * t + 1], cc2r[:, 2 * t + 1:2 * t + 2]
        nc.vector.tensor_scalar(m3[:, t:t + 1], S, 1.0 / nB, None, ALU.mult)
        nc.vector.scalar_tensor_tensor(v3[:, t:t + 1], m3[:, t:t + 1], -1.0,
                                       m3[:, t:t + 1], ALU.mult, ALU.mult)
        nc.vector.scalar_tensor_tensor(v3[:, t:t + 1], S2, 1.0 / nB,
                                       v3[:, t:t + 1], ALU.mult, ALU.add)
        nc.vector.tensor_scalar(v3[:, t:t + 1], v3[:, t:t + 1], EPS_BN, None, ALU.add)
    nc.vector.reciprocal(v3[:], v3[:])
    nc.scalar.activation(bn3S[:], v3[:], ACTF.Sqrt)
    nc.vector.tensor_mul(bn3S[:], bn3S[:], BNPK[:, 12:14])
    nc.vector.tensor_mul(m3[:], m3[:], bn3S[:])
    nc.vector.tensor_sub(bn3T[:], BNPK[:, 14:16], m3[:])

    # ---------------- relu(bn) + mlp + final residual -> out
    RELU = imgs.tile([P, 2, N2], BF16, tag="bigC")   # reuses Q slab
    out_ap = out_d.ap().rearrange("(t p) n -> p t n", p=P)
    for nn in range(8):
        nc.vector.scalar_tensor_tensor(
            RELU[:, 0, bass.ts(nn, 512)], OSB[:, 0, bass.ts(nn, 512)],
            bn3S[:, 0:1], bn3T[:, 0:1].broadcast_to([P, 512]),
            ALU.mult, ALU.add)
        nc.vector.tensor_scalar(RELU[:, 0, bass.ts(nn, 512)],
                                RELU[:, 0, bass.ts(nn, 512)], 0.0, None,
                                ALU.max)
        nc.scalar.activation(RELU[:, 1, bass.ts(nn, 512)],
                             OSB[:, 1, bass.ts(nn, 512)], ACTF.Relu,
                             bias=bn3T[:, 1:2], scale=bn3S[:, 1:2])
        for mm in range(2):
            acc = ps.tile([P, 512], FP32, tag="mm512", name=f"mlp{nn}_{mm}")
            # residual folded in as an identity matmul: runs during the
            # AllReduce flight (needs only OSB) and keeps the PE warm
            nc.tensor.matmul(acc[:], IDM[:], OSB[:, mm, bass.ts(nn, 512)],
                             start=True, stop=False)
            for kk in range(2):
                nc.tensor.matmul(acc[:], WMLP[:, kk, bass.ts(mm, P)],
                                 RELU[:, kk, bass.ts(nn, 512)],
                                 start=False, stop=(kk == 1))
            fin = tr4.tile([P, 512], FP32, tag="tr4")
            if mm == 0:
                nc.vector.tensor_copy(fin[:], acc[:])
            else:
                nc.scalar.activation(fin[:], acc[:], ACTF.Identity, bias=0.0)
            dma(out_ap[:, mm, bass.ts(nn, 512)], fin[:])

    ctx.close()


def _build_program():
    nc = bacc.Bacc("TRN2", target_bir_lowering=False, debug=False,
                   num_devices=NCORES)
    dram = {}

    def din(name, shape, dt=FP32):
        dram[name] = nc.dram_tensor(name, list(shape), dt, kind="ExternalInput")

    din("x1b", (IN_CH, N1), BF16); din("x2b", (OUT_CH, N2), BF16)
    din("wch", (IN_CH, OUT_CH), BF16); din("wkv", (IN_CH, 2 * OUT_CH), BF16)
    din("wq", (OUT_CH, OUT_CH), BF16); din("wout", (OUT_CH, OUT_CH), BF16)
    din("wmlp", (OUT_CH, OUT_CH), BF16)
    din("dw1w", (IN_CH, 9)); din("dwqw", (OUT_CH, 9)); din("dwow", (OUT_CH, 9))
    din("e1tab", (IN_CH, 9)); din("e2tab", (OUT_CH, 9))
    din("p16t", (9, NS), BF16); din("p2m", (9, 3 * 512), BF16)
    din("idm", (P, P), BF16)
    din("rt16", (N1, NS), BF16); din("r64c", (_N_R64_SLOTS * P, 512), BF16)
    din("selp", (OUT_CH, 8 * 32), BF16)
    din("bvt", (HEADS * NS, NS), BF16)
    din("gkb", (P, OUT_CH), BF16); din("bkb", (P, OUT_CH), BF16)
    din("gqb", (OUT_CH, 1), BF16); din("bqb", (OUT_CH, 1), BF16)
    din("selb", (16 * P, OUT_CH), BF16)
    din("bnpk", (P, 18))
    out_d = nc.dram_tensor("out", [OUT_CH, N2], FP32, kind="ExternalOutput")

    with tile.TileContext(nc) as tc:
        _emit(nc, tc, dram, out_d)
    nc.compile()
    return nc


# ------------------------------------------------------------------- run layer

_CACHE = {}


def _get_program():
    if "nc" not in _CACHE:
        _CACHE["nc"] = _build_program()
    return _CACHE["nc"]


def kernel(**inputs):
    nc = _get_program()
    shared = _host_prep(inputs)
    x1 = np.ascontiguousarray(np.asarray(inputs["x1"], np.float32)
                              .reshape(B, IN_CH, N1).astype(ml_dtypes.bfloat16))
    x2 = np.ascontiguousarray(np.asarray(inputs["x2"], np.float32)
                              .reshape(B, OUT_CH, N2).astype(ml_dtypes.bfloat16))
    in_maps = [dict(shared, x1b=x1[b], x2b=x2[b]) for b in range(B)]
    res = run_bass_kernel_spmd(nc, in_maps, core_ids=list(range(NCORES)))
    out = np.stack([np.asarray(res.results[b]["out"], np.float32)
                    .reshape(OUT_CH, H2, W2) for b in range(B)])
    return out
